# revision 18
# baseline (speedup 1.0000x reference)
"""Distributed RoPE multi-head attention for one TRN2 chip (8 NeuronCores).

Reference op (B=2, N=2048, C=1024, H=16, D=64, fp32):
    qkv = x @ w_qkv.T + b_qkv ; rope(q), rope(k)
    attn = softmax(q k^T / sqrt(D)) ; out = (attn v) @ w_proj.T + b_proj

Sharding: tensor-parallel over heads. Core c owns heads (2c, 2c+1) for BOTH
batch elements: it computes its slice of the QKV projection, RoPE, and full
attention for its 4 (batch, head) pairs, all in "transposed" layouts
(feature on SBUF partitions, token on the free dim) so no transposes are
needed between the matmuls. An on-chip AllToAll (2 MB/core) then reshards
the attention output from head-sharded to token-sharded, and each core runs
the output projection (full w_proj) + bias for its disjoint 512-token slice.
The host only concatenates the 8 disjoint output shards.

Matmuls run as float32r (full PE rate at free-dim >= 256, fp32 storage).
"""

import os
import sys

import numpy as np

sys.path.insert(0, "/opt/trn_rl_repo")

import ml_dtypes  # noqa: E402

BF_NP = ml_dtypes.bfloat16

import concourse.bacc as bacc  # noqa: E402
import concourse.mybir as mybir  # noqa: E402
import concourse.tile as tile  # noqa: E402

B, N, C, H, D = 2, 2048, 1024, 16, 64
T = B * N                  # 4096 flattened tokens (batch-major)
NCORES = 8
HL = H // NCORES           # 2 heads per core
CL = HL * D                # 128 local channels
TS = T // NCORES           # 512-token output slice per core
SCALE = float(D) ** -0.5
KK = C // 128              # 8 contraction tiles for the qkv matmul
KT_TILES = N // 128        # 16 key tiles per (batch, head)
QT_CH = N // 512           # 4 query chunks of 512 per batch
VS = D + 1                 # v-tile row = 64 v values + a ones column (rowsum)
VST = 80                   # per-head stride in the v tile (16B-aligned for bf16)

FP = mybir.dt.float32
FR = mybir.dt.float32r
BF = mybir.dt.bfloat16
AF = mybir.ActivationFunctionType


def _build():
    nc = bacc.Bacc(
        "TRN2",
        target_bir_lowering=False,
        debug=False,
        enable_asserts=False,
        num_devices=NCORES,
    )

    xT = nc.dram_tensor("xT", [C, T], BF, kind="ExternalInput").ap()
    wqkvT = nc.dram_tensor("wqkvT", [C, 3 * CL], BF, kind="ExternalInput").ap()
    bqkv = nc.dram_tensor("bqkv", [128, 3], FP, kind="ExternalInput").ap()
    cos2 = nc.dram_tensor("cos2", [128, T], FP, kind="ExternalInput").ap()
    sin2 = nc.dram_tensor("sin2", [128, T], FP, kind="ExternalInput").ap()
    wpT = nc.dram_tensor("wpT", [C, C], BF, kind="ExternalInput").ap()
    bproj = nc.dram_tensor("bproj", [128, 8], FP, kind="ExternalInput").ap()
    eye = nc.dram_tensor("eye", [128, 128], FP, kind="ExternalInput").ap()
    ones = nc.dram_tensor("ones", [128, T // 128 * HL], BF, kind="ExternalInput").ap()
    outT = nc.dram_tensor("outT", [C, TS], FP, kind="ExternalOutput").ap()

    with tile.TileContext(nc) as tc:
        with (
            tc.tile_pool(name="persist", bufs=1) as pp,
            tc.tile_pool(name="dram", bufs=1, space="DRAM") as dp,
        ):
            # qT/kT/vT: [2 heads x 64 feature rows, 4096 tokens]
            qT = pp.tile([128, T], BF, name="qT")
            kT = pp.tile([128, T], BF, name="kT")
            vT = pp.tile([128, T], FP, name="vT")
            # v re-tiled token-major: 32 blocks of [128 tokens, 65+65]
            # (64 v features + ones column, per head)
            vsb = pp.tile([128, (T // 128) * VST * HL], BF, name="vsb")
            eye_sb = pp.tile([128, 128], FP, name="eye_sb")
            nc.sync.dma_start(eye_sb[:], eye)

            # ---------------- Phase 1: QKV projection + RoPE + V retile ----
            with (
                tc.tile_pool(name="p1", bufs=1) as p1,
                tc.tile_pool(name="xs", bufs=1) as xs,
                tc.tile_pool(name="ps_qkv", bufs=4, space="PSUM") as ps1,
                tc.tile_pool(name="ps_tr", bufs=2, space="PSUM") as pst,
            ):
                wq = p1.tile([128, KK * 3 * CL], BF, name="wq")
                for kk in range(KK):
                    nc.sync.dma_start(
                        wq[:, kk * 3 * CL : (kk + 1) * 3 * CL],
                        wqkvT[kk * 128 : (kk + 1) * 128, :],
                    )
                bq_sb = p1.tile([128, 3], FP, name="bq_sb")
                nc.sync.dma_start(bq_sb[:], bqkv)
                cos_sb = p1.tile([128, T], FP, name="cos_sb")
                sin_sb = p1.tile([128, T], FP, name="sin_sb")
                nc.sync.dma_start(cos_sb[:], cos2)
                nc.sync.dma_start(sin_sb[:], sin2)

                xfull = []
                for kk in range(KK):
                    xf = xs.tile([128, T], BF, name="xf", tag=f"xf{kk}")
                    nc.sync.dma_start(xf[:], xT[kk * 128 : (kk + 1) * 128, :])
                    xfull.append(xf)
                qkv_dst = (qT, kT, vT)
                for t in range(T // 512):
                    sl = slice(t * 512, (t + 1) * 512)
                    for m in range(3):
                        ps = ps1.tile([128, 512], FP, name="psqkv", tag="psqkv")
                        for kk in range(KK):
                            col = kk * 3 * CL + m * 128
                            nc.tensor.matmul(
                                ps[:],
                                lhsT=wq[:, col : col + 128],
                                rhs=xfull[kk][:, sl],
                                start=(kk == 0),
                                stop=(kk == KK - 1),
                            )
                        nc.vector.tensor_scalar_add(
                            qkv_dst[m][:, sl], ps[:], bq_sb[:, m : m + 1]
                        )

                    # RoPE on this chunk of q and k, in place:
                    #   out = x*cos + rot(x)*sin_signed
                    # rot swaps the d<32 / d>=32 halves within each head's 64
                    # rows (sign folded into sin_signed host-side); partition
                    # moves must go through DMA.
                    for tgt in (qT, kT):
                        t1 = p1.tile([128, 512], FP, name="rope1", tag="rope1", bufs=3)
                        t2 = p1.tile([128, 512], BF, name="rope2", tag="rope2", bufs=3)
                        nc.vector.tensor_mul(t1[:], tgt[:, sl], cos_sb[:, sl])
                        for g in range(HL):
                            o = g * 64
                            nc.sync.dma_start(t2[o : o + 32, :], tgt[o + 32 : o + 64, sl])
                            nc.sync.dma_start(t2[o + 32 : o + 64, :], tgt[o : o + 32, sl])
                        t3 = p1.tile([128, 512], FP, name="rope3", tag="rope3", bufs=3)
                        nc.vector.tensor_mul(t3[:], t2[:], sin_sb[:, sl])
                        nc.vector.tensor_add(tgt[:, sl], t1[:], t3[:])

                    # Retile this chunk of V token-major via PE transpose
                    for tj in range(4):
                        ti = t * 4 + tj
                        pt = pst.tile([128, 128], FP, name="ptr", tag="ptr")
                        nc.tensor.transpose(
                            pt[:], vT[:, ti * 128 : (ti + 1) * 128], eye_sb[:]
                        )
                        base = ti * VST * HL
                        nc.vector.tensor_copy(vsb[:, base : base + D], pt[:, 0:D])
                        nc.vector.tensor_copy(
                            vsb[:, base + VST : base + VST + D], pt[:, D : 2 * D]
                        )
                ones_view = vsb[:].rearrange("p (t c) -> p t c", c=VST)
                nc.sync.dma_start(
                    ones_view[:, :, D : D + 1],
                    ones.rearrange("p (f o) -> p f o", o=1),
                )

            # ---------------- Phase 2: attention + AllToAll ----------------
            # One AllToAll per local head: A2A(h=0) flies while h=1's
            # attention computes, A2A(h=1) overlaps the first half of the
            # output projection.
            a2a_in = dp.tile([NCORES, CL, 512], BF, name="a2a_in")
            a2a_out = dp.tile([NCORES, CL, 512], BF, name="a2a_out")
            with (
                tc.tile_pool(name="ps_s", bufs=4, space="PSUM") as pss,
                tc.tile_pool(name="ps_o", bufs=1, space="PSUM") as pso,
                tc.tile_pool(name="exp", bufs=8) as asb,
                tc.tile_pool(name="norm", bufs=2) as nsb,
            ):
                for h in range(HL):
                    ho = h * D
                    for b in range(B):
                        pos = [
                            pso.tile([VS, 512], FP, name=f"po{qc}", tag=f"po{qc}")
                            for qc in range(QT_CH)
                        ]
                        for kt in range(KT_TILES):
                            kcol = b * N + kt * 128
                            vti = (b * N) // 128 + kt
                            vcol = vti * VST * HL + h * VST
                            exs = []
                            for qc in range(QT_CH):
                                qcol = b * N + qc * 512
                                ps = pss.tile([128, 512], FP, name="ps_s", tag="ps_s")
                                nc.tensor.matmul(
                                    ps[:],
                                    lhsT=kT[ho : ho + D, kcol : kcol + 128],
                                    rhs=qT[ho : ho + D, qcol : qcol + 512],
                                    start=True,
                                    stop=True,
                                )
                                ex = asb.tile([128, 512], BF, name="ex", tag="ex")
                                nc.scalar.activation(ex[:], ps[:], AF.Exp, scale=SCALE)
                                exs.append(ex)
                            for qc in range(QT_CH):
                                nc.tensor.matmul(
                                    pos[qc][:],
                                    lhsT=vsb[:, vcol : vcol + VS],
                                    rhs=exs[qc][:],
                                    start=(kt == 0),
                                    stop=(kt == KT_TILES - 1),
                                )
                        for qc in range(QT_CH):
                            po = pos[qc]
                            # normalize by the rowsum (last psum row) and ship
                            rc = nsb.tile([1, 512], FP, name="rc", tag="rc")
                            nc.vector.reciprocal(rc[:], po[D : D + 1, :])
                            bc = nsb.tile([D, 512], FP, name="bc", tag="bc")
                            nc.gpsimd.partition_broadcast(bc[:], rc[:])
                            an = nsb.tile([D, 512], BF, name="an", tag="an")
                            nc.vector.tensor_mul(an[:], po[0:D, :], bc[:])
                            j = b * QT_CH + qc
                            nc.sync.dma_start(
                                a2a_in[j, h * D : (h + 1) * D, :], an[:]
                            )
                nc.gpsimd.collective_compute(
                    "AllToAll",
                    mybir.AluOpType.bypass,
                    replica_groups=[list(range(NCORES))],
                    ins=[a2a_in.opt()],
                    outs=[a2a_out.opt()],
                )

            # ---------------- Phase 3: output projection -------------------
            with (
                tc.tile_pool(name="p3", bufs=1) as p3,
                tc.tile_pool(name="p3y", bufs=2) as p3y,
                tc.tile_pool(name="ps_y", bufs=8, space="PSUM") as psy,
            ):
                wp = p3.tile([128, NCORES * C], BF, name="wp")
                for j in range(NCORES):
                    nc.sync.dma_start(
                        wp[:, j * C : (j + 1) * C], wpT[j * 128 : (j + 1) * 128, :]
                    )
                bp_sb = p3.tile([128, 8], FP, name="bp_sb")
                nc.sync.dma_start(bp_sb[:], bproj)
                # gathered activations: rows 0:64 <- head-0 channels of every
                # rank, rows 64:128 <- head-1 channels (matches wp row order)
                ga = p3.tile([128, NCORES * 512], BF, name="ga")
                for j in range(NCORES):
                    nc.sync.dma_start(ga[:, j * 512 : (j + 1) * 512], a2a_out[j])
                for m in range(C // 128):
                    py = psy.tile([128, 512], FP, name="py", tag="py")
                    for j in range(NCORES):
                        col = j * C + m * 128
                        nc.tensor.matmul(
                            py[:],
                            lhsT=wp[:, col : col + 128],
                            rhs=ga[:, j * 512 : (j + 1) * 512],
                            start=(j == 0),
                            stop=(j == NCORES - 1),
                        )
                    ysb = p3y.tile([128, 512], FP, name="ysb", tag="ysb")
                    nc.scalar.activation(ysb[:], py[:], AF.Identity, bias=bp_sb[:, m : m + 1])
                    nc.sync.dma_start(outT[m * 128 : (m + 1) * 128, :], ysb[:])

    nc.compile()
    return nc


def _prep_inputs(inputs):
    """Full inputs -> per-core in_maps (all host-side, cheap reshapes)."""
    x = np.asarray(inputs["x"], dtype=np.float32)
    cos = np.asarray(inputs["cos"], dtype=np.float32)
    sin = np.asarray(inputs["sin"], dtype=np.float32)
    w_qkv = np.asarray(inputs["w_qkv"], dtype=np.float32)
    b_qkv = np.asarray(inputs["b_qkv"], dtype=np.float32)
    w_proj = np.asarray(inputs["w_proj"], dtype=np.float32)
    b_proj = np.asarray(inputs["b_proj"], dtype=np.float32)

    xT = np.ascontiguousarray(x.reshape(T, C).T).astype(BF_NP)
    cosT = cos[0, 0].T  # [64, 2048]
    sinT = sin[0, 0].T.copy()
    sinT[: D // 2] *= -1.0  # fold rotate_half's sign into sin
    cos2 = np.ascontiguousarray(np.tile(cosT, (HL, B)))
    sin2 = np.ascontiguousarray(np.tile(sinT, (HL, B)))
    wpT = np.ascontiguousarray(w_proj.T).astype(BF_NP)
    bp = np.ascontiguousarray(b_proj.reshape(8, 128).T)
    eye = np.eye(128, dtype=np.float32)

    in_maps = []
    for c in range(NCORES):
        rows = np.concatenate(
            [np.arange(g * C + c * CL, g * C + (c + 1) * CL) for g in range(3)]
        )
        wq = np.ascontiguousarray(w_qkv[rows].T).astype(BF_NP)  # [1024, 384]
        bq = np.ascontiguousarray(b_qkv[rows].reshape(3, CL).T)  # [128, 3]
        in_maps.append(
            {
                "xT": xT,
                "wqkvT": wq,
                "bqkv": bq,
                "cos2": cos2,
                "sin2": sin2,
                "wpT": wpT,
                "bproj": bp,
                "eye": eye,
                "ones": np.ones((128, T // 128 * HL), dtype=BF_NP),
            }
        )
    return in_maps


_NC_CACHE = None
last_results = None


def _install_ntff_hook():
    """Best-effort: register the axon NTFF profiling hook that the boot
    skipped (the image's antenv lacks axon_hooks). Trace-mode only."""
    try:
        import types

        if "antenv.axon_hooks" not in sys.modules:
            mod = types.ModuleType("antenv.axon_hooks")
            mod._hook = None
            mod.set_axon_ntff_profile_hook = lambda h: setattr(mod, "_hook", h)
            mod.get_axon_ntff_profile_hook = lambda: mod._hook
            sys.modules["antenv.axon_hooks"] = mod
            import antenv

            antenv.axon_hooks = mod
        import antenv.axon_hooks as ah

        if ah.get_axon_ntff_profile_hook() is None:
            if "/root/.axon_site" not in sys.path:
                sys.path.insert(0, "/root/.axon_site")
            from trn_agent_boot.trn_boot import _ntff_profile_via_ctypes

            hook = _ntff_profile_via_ctypes("/opt/axon/libaxon_pjrt.so")
            if hook is not None:
                ah.set_axon_ntff_profile_hook(hook)
        # artifact upload needs a bucket this sandbox doesn't have
        import concourse.bass_utils as bu

        bu.upload_artifacts = lambda tmpdir: tmpdir
    except Exception as e:  # pragma: no cover - profiling is optional
        print(f"ntff hook install failed: {e}", file=sys.stderr)


def kernel(**inputs):
    global _NC_CACHE, last_results
    from concourse.bass_utils import run_bass_kernel_spmd

    if _NC_CACHE is None:
        _NC_CACHE = _build()
    in_maps = _prep_inputs(inputs)
    trace = os.environ.get("KBENCH_TRACE", "0") == "1"
    if trace:
        _install_ntff_hook()
    res = run_bass_kernel_spmd(
        _NC_CACHE, in_maps, core_ids=list(range(NCORES)), trace=trace
    )
    last_results = res
    shards = [res.results[c]["outT"].T for c in range(NCORES)]  # each [512, 1024]
    y = np.concatenate(shards, axis=0).reshape(B, N, C)
    return np.ascontiguousarray(y.astype(np.float32))


# revision 21
# speedup vs baseline: 1.0277x; 1.0277x over previous
"""Distributed RoPE multi-head attention for one TRN2 chip (8 NeuronCores).

Reference op (B=2, N=2048, C=1024, H=16, D=64, fp32):
    qkv = x @ w_qkv.T + b_qkv ; rope(q), rope(k)
    attn = softmax(q k^T / sqrt(D)) ; out = (attn v) @ w_proj.T + b_proj

Sharding: tensor-parallel over heads. Core c owns heads (2c, 2c+1) for BOTH
batch elements: it computes its slice of the QKV projection, RoPE, and full
attention for its 4 (batch, head) pairs, all in "transposed" layouts
(feature on SBUF partitions, token on the free dim) so no transposes are
needed between the matmuls. An on-chip AllToAll (2 MB/core) then reshards
the attention output from head-sharded to token-sharded, and each core runs
the output projection (full w_proj) + bias for its disjoint 512-token slice.
The host only concatenates the 8 disjoint output shards.

Matmuls run as float32r (full PE rate at free-dim >= 256, fp32 storage).
"""

import os
import sys

import numpy as np

sys.path.insert(0, "/opt/trn_rl_repo")

import ml_dtypes  # noqa: E402

BF_NP = ml_dtypes.bfloat16

import concourse.bacc as bacc  # noqa: E402
import concourse.mybir as mybir  # noqa: E402
import concourse.tile as tile  # noqa: E402

B, N, C, H, D = 2, 2048, 1024, 16, 64
T = B * N                  # 4096 flattened tokens (batch-major)
NCORES = 8
HL = H // NCORES           # 2 heads per core
CL = HL * D                # 128 local channels
TS = T // NCORES           # 512-token output slice per core
SCALE = float(D) ** -0.5
KK = C // 128              # 8 contraction tiles for the qkv matmul
KT_TILES = N // 128        # 16 key tiles per (batch, head)
QT_CH = N // 512           # 4 query chunks of 512 per batch
VS = D + 1                 # v-tile row = 64 v values + a ones column (rowsum)
VST = 80                   # per-head stride in the v tile (16B-aligned for bf16)

FP = mybir.dt.float32
FR = mybir.dt.float32r
BF = mybir.dt.bfloat16
AF = mybir.ActivationFunctionType


def _build():
    nc = bacc.Bacc(
        "TRN2",
        target_bir_lowering=False,
        debug=False,
        enable_asserts=False,
        num_devices=NCORES,
    )

    xT = nc.dram_tensor("xT", [C, T], BF, kind="ExternalInput").ap()
    wqkvT = nc.dram_tensor("wqkvT", [C, 3 * CL], BF, kind="ExternalInput").ap()
    bqkv = nc.dram_tensor("bqkv", [128, 3], FP, kind="ExternalInput").ap()
    cos2 = nc.dram_tensor("cos2", [128, T], FP, kind="ExternalInput").ap()
    sin2 = nc.dram_tensor("sin2", [128, T], FP, kind="ExternalInput").ap()
    wpT = nc.dram_tensor("wpT", [C, C], BF, kind="ExternalInput").ap()
    bproj = nc.dram_tensor("bproj", [128, 8], FP, kind="ExternalInput").ap()
    eye = nc.dram_tensor("eye", [128, 128], FP, kind="ExternalInput").ap()
    ones = nc.dram_tensor("ones", [128, T // 128 * HL], BF, kind="ExternalInput").ap()
    outT = nc.dram_tensor("outT", [C, TS], FP, kind="ExternalOutput").ap()

    with tile.TileContext(nc) as tc:
        with (
            tc.tile_pool(name="persist", bufs=1) as pp,
            tc.tile_pool(name="dram", bufs=1, space="DRAM") as dp,
        ):
            # qT/kT/vT: [2 heads x 64 feature rows, 4096 tokens]
            qT = pp.tile([128, T], BF, name="qT")
            kT = pp.tile([128, T], BF, name="kT")
            vT = pp.tile([128, T], FP, name="vT")
            # v re-tiled token-major: 32 blocks of [128 tokens, 65+65]
            # (64 v features + ones column, per head)
            vsb = pp.tile([128, (T // 128) * VST * HL], BF, name="vsb")
            eye_sb = pp.tile([128, 128], FP, name="eye_sb")
            nc.sync.dma_start(eye_sb[:], eye)

            # ---------------- Phase 1: QKV projection + RoPE + V retile ----
            with (
                tc.tile_pool(name="p1", bufs=1) as p1,
                tc.tile_pool(name="xs", bufs=1) as xs,
                tc.tile_pool(name="ps_qkv", bufs=4, space="PSUM") as ps1,
                tc.tile_pool(name="ps_tr", bufs=2, space="PSUM") as pst,
            ):
                wq = p1.tile([128, KK * 3 * CL], BF, name="wq")
                for kk in range(KK):
                    nc.sync.dma_start(
                        wq[:, kk * 3 * CL : (kk + 1) * 3 * CL],
                        wqkvT[kk * 128 : (kk + 1) * 128, :],
                    )
                bq_sb = p1.tile([128, 3], FP, name="bq_sb")
                nc.sync.dma_start(bq_sb[:], bqkv)
                cos_sb = p1.tile([128, T], FP, name="cos_sb")
                sin_sb = p1.tile([128, T], FP, name="sin_sb")
                nc.sync.dma_start(cos_sb[:], cos2)
                nc.sync.dma_start(sin_sb[:], sin2)

                xfull = []
                for kk in range(KK):
                    xf = xs.tile([128, T], BF, name="xf", tag=f"xf{kk}")
                    for q4 in range(4):
                        nc.sync.dma_start(
                            xf[:, q4 * 1024 : (q4 + 1) * 1024],
                            xT[kk * 128 : (kk + 1) * 128, q4 * 1024 : (q4 + 1) * 1024],
                        )
                    xfull.append(xf)
                qkv_dst = (qT, kT, vT)
                for t in range(T // 512):
                    sl = slice(t * 512, (t + 1) * 512)
                    for m in range(3):
                        ps = ps1.tile([128, 512], FP, name="psqkv", tag="psqkv")
                        for kk in range(KK):
                            col = kk * 3 * CL + m * 128
                            nc.tensor.matmul(
                                ps[:],
                                lhsT=wq[:, col : col + 128],
                                rhs=xfull[kk][:, sl],
                                start=(kk == 0),
                                stop=(kk == KK - 1),
                            )
                        nc.vector.tensor_scalar_add(
                            qkv_dst[m][:, sl], ps[:], bq_sb[:, m : m + 1]
                        )

                    # RoPE on this chunk of q and k, in place:
                    #   out = x*cos + rot(x)*sin_signed
                    # rot swaps the d<32 / d>=32 halves within each head's 64
                    # rows (sign folded into sin_signed host-side); partition
                    # moves must go through DMA.
                    for tgt in (qT, kT):
                        t1 = p1.tile([128, 512], FP, name="rope1", tag="rope1", bufs=3)
                        t2 = p1.tile([128, 512], BF, name="rope2", tag="rope2", bufs=3)
                        nc.vector.tensor_mul(t1[:], tgt[:, sl], cos_sb[:, sl])
                        for g in range(HL):
                            o = g * 64
                            nc.sync.dma_start(t2[o : o + 32, :], tgt[o + 32 : o + 64, sl])
                            nc.sync.dma_start(t2[o + 32 : o + 64, :], tgt[o : o + 32, sl])
                        t3 = p1.tile([128, 512], FP, name="rope3", tag="rope3", bufs=3)
                        nc.vector.tensor_mul(t3[:], t2[:], sin_sb[:, sl])
                        nc.vector.tensor_add(tgt[:, sl], t1[:], t3[:])

                    # Retile this chunk of V token-major via PE transpose
                    for tj in range(4):
                        ti = t * 4 + tj
                        pt = pst.tile([128, 128], FP, name="ptr", tag="ptr")
                        nc.tensor.transpose(
                            pt[:], vT[:, ti * 128 : (ti + 1) * 128], eye_sb[:]
                        )
                        base = ti * VST * HL
                        nc.vector.tensor_copy(vsb[:, base : base + D], pt[:, 0:D])
                        nc.vector.tensor_copy(
                            vsb[:, base + VST : base + VST + D], pt[:, D : 2 * D]
                        )
                ones_view = vsb[:].rearrange("p (t c) -> p t c", c=VST)
                nc.sync.dma_start(
                    ones_view[:, :, D : D + 1],
                    ones.rearrange("p (f o) -> p f o", o=1),
                )

            # ---------------- Phase 2: attention + AllToAll ----------------
            # One AllToAll per local head: A2A(h=0) flies while h=1's
            # attention computes, A2A(h=1) overlaps the first half of the
            # output projection.
            a2a_in = [
                dp.tile([NCORES, D, 512], BF, name=f"a2a_in{h}") for h in range(HL)
            ]
            a2a_out = [
                dp.tile([NCORES, D, 512], BF, name=f"a2a_out{h}") for h in range(HL)
            ]
            with (
                tc.tile_pool(name="ps_s", bufs=4, space="PSUM") as pss,
                tc.tile_pool(name="ps_o", bufs=1, space="PSUM") as pso,
                tc.tile_pool(name="exp", bufs=8) as asb,
                tc.tile_pool(name="norm", bufs=2) as nsb,
            ):
                for h in range(HL):
                    ho = h * D
                    for b in range(B):
                        pos = [
                            pso.tile([VS, 512], FP, name=f"po{qc}", tag=f"po{qc}")
                            for qc in range(QT_CH)
                        ]
                        for kt in range(KT_TILES):
                            kcol = b * N + kt * 128
                            vti = (b * N) // 128 + kt
                            vcol = vti * VST * HL + h * VST
                            exs = []
                            for qc in range(QT_CH):
                                qcol = b * N + qc * 512
                                ps = pss.tile([128, 512], FP, name="ps_s", tag="ps_s")
                                nc.tensor.matmul(
                                    ps[:],
                                    lhsT=kT[ho : ho + D, kcol : kcol + 128],
                                    rhs=qT[ho : ho + D, qcol : qcol + 512],
                                    start=True,
                                    stop=True,
                                )
                                ex = asb.tile([128, 512], BF, name="ex", tag="ex")
                                nc.scalar.activation(ex[:], ps[:], AF.Exp, scale=SCALE)
                                exs.append(ex)
                            for qc in range(QT_CH):
                                nc.tensor.matmul(
                                    pos[qc][:],
                                    lhsT=vsb[:, vcol : vcol + VS],
                                    rhs=exs[qc][:],
                                    start=(kt == 0),
                                    stop=(kt == KT_TILES - 1),
                                )
                        for qc in range(QT_CH):
                            # evict psum first so the bank frees for the next
                            # (h, b) pair, then normalize from SBUF
                            poc = nsb.tile([VS, 512], FP, name="poc", tag="poc", bufs=4)
                            nc.vector.tensor_copy(poc[:], pos[qc][:])
                            rc = nsb.tile([1, 512], FP, name="rc", tag="rc")
                            nc.vector.reciprocal(rc[:], poc[D : D + 1, :])
                            bc = nsb.tile([D, 512], FP, name="bc", tag="bc")
                            nc.gpsimd.partition_broadcast(bc[:], rc[:])
                            an = nsb.tile([D, 512], BF, name="an", tag="an")
                            nc.vector.tensor_mul(an[:], poc[0:D, :], bc[:])
                            j = b * QT_CH + qc
                            nc.sync.dma_start(a2a_in[h][j, :, :], an[:])
                    nc.gpsimd.collective_compute(
                        "AllToAll",
                        mybir.AluOpType.bypass,
                        replica_groups=[list(range(NCORES))],
                        ins=[a2a_in[h].opt()],
                        outs=[a2a_out[h].opt()],
                    )

            # ---------------- Phase 3: output projection -------------------
            with (
                tc.tile_pool(name="p3", bufs=1) as p3,
                tc.tile_pool(name="p3y", bufs=2) as p3y,
                tc.tile_pool(name="ps_y", bufs=8, space="PSUM") as psy,
            ):
                wp = p3.tile([128, NCORES * C], BF, name="wp")
                for j in range(NCORES):
                    nc.sync.dma_start(
                        wp[:, j * C : (j + 1) * C], wpT[j * 128 : (j + 1) * 128, :]
                    )
                bp_sb = p3.tile([128, 8], FP, name="bp_sb")
                nc.sync.dma_start(bp_sb[:], bproj)
                # gathered activations: rows 0:64 <- head-0 channels of every
                # rank, rows 64:128 <- head-1 channels (matches wp row order)
                # ga rows 0:64 <- head-0 channels of every rank, 64:128 <- head-1
                ga = p3.tile([128, NCORES * 512], BF, name="ga")
                for h in range(HL):
                    for j in range(NCORES):
                        nc.sync.dma_start(
                            ga[h * D : (h + 1) * D, j * 512 : (j + 1) * 512],
                            a2a_out[h][j],
                        )
                for m in range(C // 128):
                    py = psy.tile([128, 512], FP, name="py", tag="py")
                    for j in range(NCORES):
                        col = j * C + m * 128
                        nc.tensor.matmul(
                            py[:],
                            lhsT=wp[:, col : col + 128],
                            rhs=ga[:, j * 512 : (j + 1) * 512],
                            start=(j == 0),
                            stop=(j == NCORES - 1),
                        )
                    ysb = p3y.tile([128, 512], FP, name="ysb", tag="ysb")
                    nc.scalar.activation(ysb[:], py[:], AF.Identity, bias=bp_sb[:, m : m + 1])
                    nc.sync.dma_start(outT[m * 128 : (m + 1) * 128, :], ysb[:])

    nc.compile()
    return nc


def _prep_inputs(inputs):
    """Full inputs -> per-core in_maps (all host-side, cheap reshapes)."""
    x = np.asarray(inputs["x"], dtype=np.float32)
    cos = np.asarray(inputs["cos"], dtype=np.float32)
    sin = np.asarray(inputs["sin"], dtype=np.float32)
    w_qkv = np.asarray(inputs["w_qkv"], dtype=np.float32)
    b_qkv = np.asarray(inputs["b_qkv"], dtype=np.float32)
    w_proj = np.asarray(inputs["w_proj"], dtype=np.float32)
    b_proj = np.asarray(inputs["b_proj"], dtype=np.float32)

    xT = np.ascontiguousarray(x.reshape(T, C).T).astype(BF_NP)
    cosT = cos[0, 0].T  # [64, 2048]
    sinT = sin[0, 0].T.copy()
    sinT[: D // 2] *= -1.0  # fold rotate_half's sign into sin
    cos2 = np.ascontiguousarray(np.tile(cosT, (HL, B)))
    sin2 = np.ascontiguousarray(np.tile(sinT, (HL, B)))
    wpT = np.ascontiguousarray(w_proj.T).astype(BF_NP)
    bp = np.ascontiguousarray(b_proj.reshape(8, 128).T)
    eye = np.eye(128, dtype=np.float32)

    in_maps = []
    for c in range(NCORES):
        rows = np.concatenate(
            [np.arange(g * C + c * CL, g * C + (c + 1) * CL) for g in range(3)]
        )
        wq = np.ascontiguousarray(w_qkv[rows].T).astype(BF_NP)  # [1024, 384]
        bq = np.ascontiguousarray(b_qkv[rows].reshape(3, CL).T)  # [128, 3]
        in_maps.append(
            {
                "xT": xT,
                "wqkvT": wq,
                "bqkv": bq,
                "cos2": cos2,
                "sin2": sin2,
                "wpT": wpT,
                "bproj": bp,
                "eye": eye,
                "ones": np.ones((128, T // 128 * HL), dtype=BF_NP),
            }
        )
    return in_maps


_NC_CACHE = None
last_results = None


def _install_ntff_hook():
    """Best-effort: register the axon NTFF profiling hook that the boot
    skipped (the image's antenv lacks axon_hooks). Trace-mode only."""
    try:
        import types

        if "antenv.axon_hooks" not in sys.modules:
            mod = types.ModuleType("antenv.axon_hooks")
            mod._hook = None
            mod.set_axon_ntff_profile_hook = lambda h: setattr(mod, "_hook", h)
            mod.get_axon_ntff_profile_hook = lambda: mod._hook
            sys.modules["antenv.axon_hooks"] = mod
            import antenv

            antenv.axon_hooks = mod
        import antenv.axon_hooks as ah

        if ah.get_axon_ntff_profile_hook() is None:
            if "/root/.axon_site" not in sys.path:
                sys.path.insert(0, "/root/.axon_site")
            from trn_agent_boot.trn_boot import _ntff_profile_via_ctypes

            hook = _ntff_profile_via_ctypes("/opt/axon/libaxon_pjrt.so")
            if hook is not None:
                ah.set_axon_ntff_profile_hook(hook)
        # artifact upload needs a bucket this sandbox doesn't have
        import concourse.bass_utils as bu

        bu.upload_artifacts = lambda tmpdir: tmpdir
    except Exception as e:  # pragma: no cover - profiling is optional
        print(f"ntff hook install failed: {e}", file=sys.stderr)


def kernel(**inputs):
    global _NC_CACHE, last_results
    from concourse.bass_utils import run_bass_kernel_spmd

    if _NC_CACHE is None:
        _NC_CACHE = _build()
    in_maps = _prep_inputs(inputs)
    trace = os.environ.get("KBENCH_TRACE", "0") == "1"
    if trace:
        _install_ntff_hook()
    res = run_bass_kernel_spmd(
        _NC_CACHE, in_maps, core_ids=list(range(NCORES)), trace=trace
    )
    last_results = res
    shards = [res.results[c]["outT"].T for c in range(NCORES)]  # each [512, 1024]
    y = np.concatenate(shards, axis=0).reshape(B, N, C)
    return np.ascontiguousarray(y.astype(np.float32))


# revision 23
# speedup vs baseline: 1.0594x; 1.0309x over previous
"""Distributed RoPE multi-head attention for one TRN2 chip (8 NeuronCores).

Reference op (B=2, N=2048, C=1024, H=16, D=64, fp32):
    qkv = x @ w_qkv.T + b_qkv ; rope(q), rope(k)
    attn = softmax(q k^T / sqrt(D)) ; out = (attn v) @ w_proj.T + b_proj

Sharding: tensor-parallel over heads. Core c owns heads (2c, 2c+1) for BOTH
batch elements: it computes its slice of the QKV projection, RoPE, and full
attention for its 4 (batch, head) pairs, all in "transposed" layouts
(feature on SBUF partitions, token on the free dim) so no transposes are
needed between the matmuls. An on-chip AllToAll (2 MB/core) then reshards
the attention output from head-sharded to token-sharded, and each core runs
the output projection (full w_proj) + bias for its disjoint 512-token slice.
The host only concatenates the 8 disjoint output shards.

Matmuls run as float32r (full PE rate at free-dim >= 256, fp32 storage).
"""

import os
import sys

import numpy as np

sys.path.insert(0, "/opt/trn_rl_repo")

import ml_dtypes  # noqa: E402

BF_NP = ml_dtypes.bfloat16

import concourse.bacc as bacc  # noqa: E402
import concourse.mybir as mybir  # noqa: E402
import concourse.tile as tile  # noqa: E402

B, N, C, H, D = 2, 2048, 1024, 16, 64
T = B * N                  # 4096 flattened tokens (batch-major)
NCORES = 8
HL = H // NCORES           # 2 heads per core
CL = HL * D                # 128 local channels
TS = T // NCORES           # 512-token output slice per core
SCALE = float(D) ** -0.5
KK = C // 128              # 8 contraction tiles for the qkv matmul
KT_TILES = N // 128        # 16 key tiles per (batch, head)
QT_CH = N // 512           # 4 query chunks of 512 per batch
VS = D + 1                 # v-tile row = 64 v values + a ones column (rowsum)
VST = 80                   # per-head stride in the v tile (16B-aligned for bf16)

FP = mybir.dt.float32
FR = mybir.dt.float32r
BF = mybir.dt.bfloat16
AF = mybir.ActivationFunctionType


def _build():
    nc = bacc.Bacc(
        "TRN2",
        target_bir_lowering=False,
        debug=False,
        enable_asserts=False,
        num_devices=NCORES,
    )

    xT = nc.dram_tensor("xT", [C, T], BF, kind="ExternalInput").ap()
    wqkvT = nc.dram_tensor("wqkvT", [C, 3 * CL], BF, kind="ExternalInput").ap()
    bqkv = nc.dram_tensor("bqkv", [128, 3], FP, kind="ExternalInput").ap()
    cos2 = nc.dram_tensor("cos2", [128, T], BF, kind="ExternalInput").ap()
    sin2 = nc.dram_tensor("sin2", [128, T], BF, kind="ExternalInput").ap()
    wpT = nc.dram_tensor("wpT", [C, C], BF, kind="ExternalInput").ap()
    bproj = nc.dram_tensor("bproj", [128, 8], FP, kind="ExternalInput").ap()
    eye = nc.dram_tensor("eye", [128, 128], FP, kind="ExternalInput").ap()
    ones = nc.dram_tensor("ones", [128, T // 128 * HL], BF, kind="ExternalInput").ap()
    outT = nc.dram_tensor("outT", [C, TS], FP, kind="ExternalOutput").ap()

    with tile.TileContext(nc) as tc:
        with (
            tc.tile_pool(name="persist", bufs=1) as pp,
            tc.tile_pool(name="dram", bufs=1, space="DRAM") as dp,
        ):
            # qT/kT/vT: [2 heads x 64 feature rows, 4096 tokens]
            qT = pp.tile([128, T], BF, name="qT")
            kT = pp.tile([128, T], BF, name="kT")
            vT = pp.tile([128, T], FP, name="vT")
            # v re-tiled token-major: 32 blocks of [128 tokens, 65+65]
            # (64 v features + ones column, per head)
            vsb = pp.tile([128, (T // 128) * VST * HL], BF, name="vsb")
            eye_sb = pp.tile([128, 128], FP, name="eye_sb")
            nc.sync.dma_start(eye_sb[:], eye)

            # ---------------- Phase 1: QKV projection + RoPE + V retile ----
            with (
                tc.tile_pool(name="p1", bufs=1) as p1,
                tc.tile_pool(name="xs", bufs=1) as xs,
                tc.tile_pool(name="ps_qkv", bufs=4, space="PSUM") as ps1,
                tc.tile_pool(name="ps_tr", bufs=2, space="PSUM") as pst,
            ):
                wq = p1.tile([128, KK * 3 * CL], BF, name="wq")
                for kk in range(KK):
                    nc.sync.dma_start(
                        wq[:, kk * 3 * CL : (kk + 1) * 3 * CL],
                        wqkvT[kk * 128 : (kk + 1) * 128, :],
                    )
                bq_sb = p1.tile([128, 3], FP, name="bq_sb")
                nc.sync.dma_start(bq_sb[:], bqkv)

                xfull = []
                for kk in range(KK):
                    xf = xs.tile([128, T], BF, name="xf", tag=f"xf{kk}")
                    for q4 in range(4):
                        nc.sync.dma_start(
                            xf[:, q4 * 1024 : (q4 + 1) * 1024],
                            xT[kk * 128 : (kk + 1) * 128, q4 * 1024 : (q4 + 1) * 1024],
                        )
                    xfull.append(xf)
                cos_sb = p1.tile([128, T], BF, name="cos_sb")
                sin_sb = p1.tile([128, T], BF, name="sin_sb")
                for q4 in range(8):
                    qsl = slice(q4 * 512, (q4 + 1) * 512)
                    nc.sync.dma_start(cos_sb[:, qsl], cos2[:, qsl])
                    nc.sync.dma_start(sin_sb[:, qsl], sin2[:, qsl])
                qkv_dst = (qT, kT, vT)
                for t in range(T // 512):
                    sl = slice(t * 512, (t + 1) * 512)
                    for m in range(3):
                        ps = ps1.tile([128, 512], FP, name="psqkv", tag="psqkv")
                        for kk in range(KK):
                            col = kk * 3 * CL + m * 128
                            nc.tensor.matmul(
                                ps[:],
                                lhsT=wq[:, col : col + 128],
                                rhs=xfull[kk][:, sl],
                                start=(kk == 0),
                                stop=(kk == KK - 1),
                            )
                        nc.vector.tensor_scalar_add(
                            qkv_dst[m][:, sl], ps[:], bq_sb[:, m : m + 1]
                        )

                    # RoPE on this chunk of q and k, in place:
                    #   out = x*cos + rot(x)*sin_signed
                    # rot swaps the d<32 / d>=32 halves within each head's 64
                    # rows (sign folded into sin_signed host-side); partition
                    # moves must go through DMA.
                    for tgt in (qT, kT):
                        t1 = p1.tile([128, 512], FP, name="rope1", tag="rope1", bufs=3)
                        t2 = p1.tile([128, 512], BF, name="rope2", tag="rope2", bufs=3)
                        nc.vector.tensor_mul(t1[:], tgt[:, sl], cos_sb[:, sl])
                        for g in range(HL):
                            o = g * 64
                            nc.sync.dma_start(t2[o : o + 32, :], tgt[o + 32 : o + 64, sl])
                            nc.sync.dma_start(t2[o + 32 : o + 64, :], tgt[o : o + 32, sl])
                        t3 = p1.tile([128, 512], FP, name="rope3", tag="rope3", bufs=3)
                        nc.vector.tensor_mul(t3[:], t2[:], sin_sb[:, sl])
                        nc.vector.tensor_add(tgt[:, sl], t1[:], t3[:])

                    # Retile this chunk of V token-major via PE transpose
                    for tj in range(4):
                        ti = t * 4 + tj
                        pt = pst.tile([128, 128], FP, name="ptr", tag="ptr")
                        nc.tensor.transpose(
                            pt[:], vT[:, ti * 128 : (ti + 1) * 128], eye_sb[:]
                        )
                        base = ti * VST * HL
                        nc.vector.tensor_copy(vsb[:, base : base + D], pt[:, 0:D])
                        nc.vector.tensor_copy(
                            vsb[:, base + VST : base + VST + D], pt[:, D : 2 * D]
                        )
                ones_view = vsb[:].rearrange("p (t c) -> p t c", c=VST)
                nc.sync.dma_start(
                    ones_view[:, :, D : D + 1],
                    ones.rearrange("p (f o) -> p f o", o=1),
                )

            # ---------------- Phase 2: attention + AllToAll ----------------
            # One AllToAll per local head: A2A(h=0) flies while h=1's
            # attention computes, A2A(h=1) overlaps the first half of the
            # output projection.
            a2a_in = [
                dp.tile([NCORES, D, 512], BF, name=f"a2a_in{h}") for h in range(HL)
            ]
            a2a_out = [
                dp.tile([NCORES, D, 512], BF, name=f"a2a_out{h}") for h in range(HL)
            ]
            with (
                tc.tile_pool(name="ps_s", bufs=4, space="PSUM") as pss,
                tc.tile_pool(name="ps_o", bufs=1, space="PSUM") as pso,
                tc.tile_pool(name="exp", bufs=8) as asb,
                tc.tile_pool(name="norm", bufs=2) as nsb,
            ):
                for h in range(HL):
                    ho = h * D
                    for b in range(B):
                        pos = [
                            pso.tile([VS, 512], FP, name=f"po{qc}", tag=f"po{qc}")
                            for qc in range(QT_CH)
                        ]
                        for kt in range(KT_TILES):
                            kcol = b * N + kt * 128
                            vti = (b * N) // 128 + kt
                            vcol = vti * VST * HL + h * VST
                            exs = []
                            for qc in range(QT_CH):
                                qcol = b * N + qc * 512
                                ps = pss.tile([128, 512], FP, name="ps_s", tag="ps_s")
                                nc.tensor.matmul(
                                    ps[:],
                                    lhsT=kT[ho : ho + D, kcol : kcol + 128],
                                    rhs=qT[ho : ho + D, qcol : qcol + 512],
                                    start=True,
                                    stop=True,
                                )
                                ex = asb.tile([128, 512], BF, name="ex", tag="ex")
                                nc.scalar.activation(ex[:], ps[:], AF.Exp, scale=SCALE)
                                exs.append(ex)
                            for qc in range(QT_CH):
                                nc.tensor.matmul(
                                    pos[qc][:],
                                    lhsT=vsb[:, vcol : vcol + VS],
                                    rhs=exs[qc][:],
                                    start=(kt == 0),
                                    stop=(kt == KT_TILES - 1),
                                )
                        for qc in range(QT_CH):
                            # evict psum first so the bank frees for the next
                            # (h, b) pair, then normalize from SBUF
                            poc = nsb.tile([VS, 512], FP, name="poc", tag="poc", bufs=4)
                            nc.vector.tensor_copy(poc[:], pos[qc][:])
                            rc = nsb.tile([1, 512], FP, name="rc", tag="rc")
                            nc.vector.reciprocal(rc[:], poc[D : D + 1, :])
                            bc = nsb.tile([D, 512], FP, name="bc", tag="bc")
                            nc.gpsimd.partition_broadcast(bc[:], rc[:])
                            an = nsb.tile([D, 512], BF, name="an", tag="an")
                            nc.vector.tensor_mul(an[:], poc[0:D, :], bc[:])
                            j = b * QT_CH + qc
                            nc.sync.dma_start(a2a_in[h][j, :, :], an[:])
                    nc.gpsimd.collective_compute(
                        "AllToAll",
                        mybir.AluOpType.bypass,
                        replica_groups=[list(range(NCORES))],
                        ins=[a2a_in[h].opt()],
                        outs=[a2a_out[h].opt()],
                    )

            # ---------------- Phase 3: output projection -------------------
            with (
                tc.tile_pool(name="p3", bufs=1) as p3,
                tc.tile_pool(name="p3y", bufs=2) as p3y,
                tc.tile_pool(name="ps_y", bufs=2, space="PSUM") as psy,
            ):
                wp = p3.tile([128, NCORES * C], BF, name="wp")
                for j in range(NCORES):
                    nc.sync.dma_start(
                        wp[:, j * C : (j + 1) * C], wpT[j * 128 : (j + 1) * 128, :]
                    )
                bp_sb = p3.tile([128, 8], FP, name="bp_sb")
                nc.sync.dma_start(bp_sb[:], bproj)
                # gathered activations: rows 0:64 <- head-0 channels of every
                # rank, rows 64:128 <- head-1 channels (matches wp row order)
                # ga rows 0:64 <- head-0 channels of every rank, 64:128 <- head-1
                ga = p3.tile([128, NCORES * 512], BF, name="ga")
                for h in range(HL):
                    for j in range(NCORES):
                        nc.sync.dma_start(
                            ga[h * D : (h + 1) * D, j * 512 : (j + 1) * 512],
                            a2a_out[h][j],
                        )
                # two independent K=64 accumulation groups per m-tile: the
                # h=0 group closes before A2A(h=1) lands, so it overlaps comm;
                # DVE combines the halves and adds the bias.
                for m in range(C // 128):
                    pys = []
                    for h in range(HL):
                        py = psy.tile([128, 512], FP, name=f"py{h}", tag=f"py{h}")
                        for j in range(NCORES):
                            col = j * C + m * 128
                            nc.tensor.matmul(
                                py[:],
                                lhsT=wp[h * D : (h + 1) * D, col : col + 128],
                                rhs=ga[h * D : (h + 1) * D, j * 512 : (j + 1) * 512],
                                start=(j == 0),
                                stop=(j == NCORES - 1),
                            )
                        pys.append(py)
                    y0 = p3y.tile([128, 512], FP, name="y0", tag="y0")
                    nc.vector.tensor_copy(y0[:], pys[0][:])
                    ysb = p3y.tile([128, 512], FP, name="ysb", tag="ysb")
                    nc.vector.scalar_tensor_tensor(
                        ysb[:], pys[1][:], bp_sb[:, m : m + 1], y0[:],
                        op0=mybir.AluOpType.add, op1=mybir.AluOpType.add,
                    )
                    nc.sync.dma_start(outT[m * 128 : (m + 1) * 128, :], ysb[:])

    nc.compile()
    return nc


def _prep_inputs(inputs):
    """Full inputs -> per-core in_maps (all host-side, cheap reshapes)."""
    x = np.asarray(inputs["x"], dtype=np.float32)
    cos = np.asarray(inputs["cos"], dtype=np.float32)
    sin = np.asarray(inputs["sin"], dtype=np.float32)
    w_qkv = np.asarray(inputs["w_qkv"], dtype=np.float32)
    b_qkv = np.asarray(inputs["b_qkv"], dtype=np.float32)
    w_proj = np.asarray(inputs["w_proj"], dtype=np.float32)
    b_proj = np.asarray(inputs["b_proj"], dtype=np.float32)

    xT = np.ascontiguousarray(x.reshape(T, C).T).astype(BF_NP)
    cosT = cos[0, 0].T  # [64, 2048]
    sinT = sin[0, 0].T.copy()
    sinT[: D // 2] *= -1.0  # fold rotate_half's sign into sin
    cos2 = np.ascontiguousarray(np.tile(cosT, (HL, B))).astype(BF_NP)
    sin2 = np.ascontiguousarray(np.tile(sinT, (HL, B))).astype(BF_NP)
    wpT = np.ascontiguousarray(w_proj.T).astype(BF_NP)
    bp = np.ascontiguousarray(b_proj.reshape(8, 128).T)
    eye = np.eye(128, dtype=np.float32)

    in_maps = []
    for c in range(NCORES):
        rows = np.concatenate(
            [np.arange(g * C + c * CL, g * C + (c + 1) * CL) for g in range(3)]
        )
        wq = np.ascontiguousarray(w_qkv[rows].T).astype(BF_NP)  # [1024, 384]
        bq = np.ascontiguousarray(b_qkv[rows].reshape(3, CL).T)  # [128, 3]
        in_maps.append(
            {
                "xT": xT,
                "wqkvT": wq,
                "bqkv": bq,
                "cos2": cos2,
                "sin2": sin2,
                "wpT": wpT,
                "bproj": bp,
                "eye": eye,
                "ones": np.ones((128, T // 128 * HL), dtype=BF_NP),
            }
        )
    return in_maps


_NC_CACHE = None
last_results = None


def _install_ntff_hook():
    """Best-effort: register the axon NTFF profiling hook that the boot
    skipped (the image's antenv lacks axon_hooks). Trace-mode only."""
    try:
        import types

        if "antenv.axon_hooks" not in sys.modules:
            mod = types.ModuleType("antenv.axon_hooks")
            mod._hook = None
            mod.set_axon_ntff_profile_hook = lambda h: setattr(mod, "_hook", h)
            mod.get_axon_ntff_profile_hook = lambda: mod._hook
            sys.modules["antenv.axon_hooks"] = mod
            import antenv

            antenv.axon_hooks = mod
        import antenv.axon_hooks as ah

        if ah.get_axon_ntff_profile_hook() is None:
            if "/root/.axon_site" not in sys.path:
                sys.path.insert(0, "/root/.axon_site")
            from trn_agent_boot.trn_boot import _ntff_profile_via_ctypes

            hook = _ntff_profile_via_ctypes("/opt/axon/libaxon_pjrt.so")
            if hook is not None:
                ah.set_axon_ntff_profile_hook(hook)
        # artifact upload needs a bucket this sandbox doesn't have
        import concourse.bass_utils as bu

        bu.upload_artifacts = lambda tmpdir: tmpdir
    except Exception as e:  # pragma: no cover - profiling is optional
        print(f"ntff hook install failed: {e}", file=sys.stderr)


def kernel(**inputs):
    global _NC_CACHE, last_results
    from concourse.bass_utils import run_bass_kernel_spmd

    if _NC_CACHE is None:
        _NC_CACHE = _build()
    in_maps = _prep_inputs(inputs)
    trace = os.environ.get("KBENCH_TRACE", "0") == "1"
    if trace:
        _install_ntff_hook()
    res = run_bass_kernel_spmd(
        _NC_CACHE, in_maps, core_ids=list(range(NCORES)), trace=trace
    )
    last_results = res
    shards = [res.results[c]["outT"].T for c in range(NCORES)]  # each [512, 1024]
    y = np.concatenate(shards, axis=0).reshape(B, N, C)
    return np.ascontiguousarray(y.astype(np.float32))


# revision 25
# speedup vs baseline: 1.0693x; 1.0094x over previous
"""Distributed RoPE multi-head attention for one TRN2 chip (8 NeuronCores).

Reference op (B=2, N=2048, C=1024, H=16, D=64, fp32):
    qkv = x @ w_qkv.T + b_qkv ; rope(q), rope(k)
    attn = softmax(q k^T / sqrt(D)) ; out = (attn v) @ w_proj.T + b_proj

Sharding: tensor-parallel over heads. Core c owns heads (2c, 2c+1) for BOTH
batch elements: it computes its slice of the QKV projection, RoPE, and full
attention for its 4 (batch, head) pairs, all in "transposed" layouts
(feature on SBUF partitions, token on the free dim) so no transposes are
needed between the matmuls. An on-chip AllToAll (2 MB/core) then reshards
the attention output from head-sharded to token-sharded, and each core runs
the output projection (full w_proj) + bias for its disjoint 512-token slice.
The host only concatenates the 8 disjoint output shards.

Matmuls run as float32r (full PE rate at free-dim >= 256, fp32 storage).
"""

import os
import sys

import numpy as np

sys.path.insert(0, "/opt/trn_rl_repo")

import ml_dtypes  # noqa: E402

BF_NP = ml_dtypes.bfloat16

import concourse.bacc as bacc  # noqa: E402
import concourse.mybir as mybir  # noqa: E402
import concourse.tile as tile  # noqa: E402

B, N, C, H, D = 2, 2048, 1024, 16, 64
T = B * N                  # 4096 flattened tokens (batch-major)
NCORES = 8
HL = H // NCORES           # 2 heads per core
CL = HL * D                # 128 local channels
TS = T // NCORES           # 512-token output slice per core
SCALE = float(D) ** -0.5
KK = C // 128              # 8 contraction tiles for the qkv matmul
KT_TILES = N // 128        # 16 key tiles per (batch, head)
QT_CH = N // 512           # 4 query chunks of 512 per batch
VS = D + 1                 # v-tile row = 64 v values + a ones column (rowsum)
VST = 80                   # per-head stride in the v tile (16B-aligned for bf16)

FP = mybir.dt.float32
FR = mybir.dt.float32r
BF = mybir.dt.bfloat16
AF = mybir.ActivationFunctionType


def _build():
    nc = bacc.Bacc(
        "TRN2",
        target_bir_lowering=False,
        debug=False,
        enable_asserts=False,
        num_devices=NCORES,
    )

    xT = nc.dram_tensor("xT", [C, T], BF, kind="ExternalInput").ap()
    wqkvT = nc.dram_tensor("wqkvT", [C, 3 * CL], BF, kind="ExternalInput").ap()
    bqkv = nc.dram_tensor("bqkv", [128, 3], FP, kind="ExternalInput").ap()
    cos2 = nc.dram_tensor("cos2", [128, T], BF, kind="ExternalInput").ap()
    sin2 = nc.dram_tensor("sin2", [128, T], BF, kind="ExternalInput").ap()
    wpT = nc.dram_tensor("wpT", [C, C], BF, kind="ExternalInput").ap()
    bproj = nc.dram_tensor("bproj", [128, 8], FP, kind="ExternalInput").ap()
    eye = nc.dram_tensor("eye", [128, 128], FP, kind="ExternalInput").ap()
    ones = nc.dram_tensor("ones", [128, T // 128 * HL], BF, kind="ExternalInput").ap()
    outT = nc.dram_tensor("outT", [C, TS], FP, kind="ExternalOutput").ap()

    with tile.TileContext(nc) as tc:
        with (
            tc.tile_pool(name="persist", bufs=1) as pp,
            tc.tile_pool(name="dram", bufs=1, space="DRAM") as dp,
        ):
            # qT/kT/vT: [2 heads x 64 feature rows, 4096 tokens]
            qT = pp.tile([128, T], BF, name="qT")
            kT = pp.tile([128, T], BF, name="kT")
            vT = pp.tile([128, T], FP, name="vT")
            # v re-tiled token-major: 32 blocks of [128 tokens, 65+65]
            # (64 v features + ones column, per head)
            vsb = pp.tile([128, (T // 128) * VST * HL], BF, name="vsb")
            eye_sb = pp.tile([128, 128], FP, name="eye_sb")
            nc.sync.dma_start(eye_sb[:], eye)

            # ---------------- Phase 1: QKV projection + RoPE + V retile ----
            with (
                tc.tile_pool(name="p1", bufs=1) as p1,
                tc.tile_pool(name="xs", bufs=1) as xs,
                tc.tile_pool(name="ps_qkv", bufs=4, space="PSUM") as ps1,
                tc.tile_pool(name="ps_tr", bufs=2, space="PSUM") as pst,
            ):
                wq = p1.tile([128, KK * 3 * CL], BF, name="wq")
                for kk in range(KK):
                    nc.sync.dma_start(
                        wq[:, kk * 3 * CL : (kk + 1) * 3 * CL],
                        wqkvT[kk * 128 : (kk + 1) * 128, :],
                    )
                bq_sb = p1.tile([128, 3], FP, name="bq_sb")
                nc.sync.dma_start(bq_sb[:], bqkv)

                dma_engines = (nc.sync, nc.gpsimd, nc.scalar)
                xfull = []
                for kk in range(KK):
                    xf = xs.tile([128, T], BF, name="xf", tag=f"xf{kk}")
                    # contiguous 1MB read, spread across engine DMA queues
                    dma_engines[kk % 3].dma_start(
                        xf[:], xT[kk * 128 : (kk + 1) * 128, :]
                    )
                    xfull.append(xf)
                cos_sb = p1.tile([128, T], BF, name="cos_sb")
                sin_sb = p1.tile([128, T], BF, name="sin_sb")
                nc.gpsimd.dma_start(cos_sb[:], cos2)
                nc.scalar.dma_start(sin_sb[:], sin2)
                qkv_dst = (qT, kT, vT)
                for t in range(T // 512):
                    sl = slice(t * 512, (t + 1) * 512)
                    for m in range(3):
                        ps = ps1.tile([128, 512], FP, name="psqkv", tag="psqkv")
                        for kk in range(KK):
                            col = kk * 3 * CL + m * 128
                            nc.tensor.matmul(
                                ps[:],
                                lhsT=wq[:, col : col + 128],
                                rhs=xfull[kk][:, sl],
                                start=(kk == 0),
                                stop=(kk == KK - 1),
                            )
                        nc.vector.tensor_scalar_add(
                            qkv_dst[m][:, sl], ps[:], bq_sb[:, m : m + 1]
                        )

                    # RoPE on this chunk of q and k, in place:
                    #   out = x*cos + rot(x)*sin_signed
                    # rot swaps the d<32 / d>=32 halves within each head's 64
                    # rows (sign folded into sin_signed host-side); partition
                    # moves must go through DMA.
                    for tgt in (qT, kT):
                        t1 = p1.tile([128, 512], FP, name="rope1", tag="rope1", bufs=3)
                        t2 = p1.tile([128, 512], BF, name="rope2", tag="rope2", bufs=3)
                        nc.vector.tensor_mul(t1[:], tgt[:, sl], cos_sb[:, sl])
                        for g in range(HL):
                            o = g * 64
                            nc.sync.dma_start(t2[o : o + 32, :], tgt[o + 32 : o + 64, sl])
                            nc.sync.dma_start(t2[o + 32 : o + 64, :], tgt[o : o + 32, sl])
                        t3 = p1.tile([128, 512], FP, name="rope3", tag="rope3", bufs=3)
                        nc.vector.tensor_mul(t3[:], t2[:], sin_sb[:, sl])
                        nc.vector.tensor_add(tgt[:, sl], t1[:], t3[:])

                    # Retile this chunk of V token-major via PE transpose
                    for tj in range(4):
                        ti = t * 4 + tj
                        pt = pst.tile([128, 128], FP, name="ptr", tag="ptr")
                        nc.tensor.transpose(
                            pt[:], vT[:, ti * 128 : (ti + 1) * 128], eye_sb[:]
                        )
                        base = ti * VST * HL
                        nc.vector.tensor_copy(vsb[:, base : base + D], pt[:, 0:D])
                        nc.vector.tensor_copy(
                            vsb[:, base + VST : base + VST + D], pt[:, D : 2 * D]
                        )
                ones_view = vsb[:].rearrange("p (t c) -> p t c", c=VST)
                nc.sync.dma_start(
                    ones_view[:, :, D : D + 1],
                    ones.rearrange("p (f o) -> p f o", o=1),
                )

            # ---------------- Phase 2: attention + AllToAll ----------------
            # One AllToAll per local head: A2A(h=0) flies while h=1's
            # attention computes, A2A(h=1) overlaps the first half of the
            # output projection.
            a2a_in = [
                dp.tile([NCORES, D, 512], BF, name=f"a2a_in{h}") for h in range(HL)
            ]
            a2a_out = [
                dp.tile([NCORES, D, 512], BF, name=f"a2a_out{h}") for h in range(HL)
            ]
            with (
                tc.tile_pool(name="ps_s", bufs=4, space="PSUM") as pss,
                tc.tile_pool(name="ps_o", bufs=1, space="PSUM") as pso,
                tc.tile_pool(name="exp", bufs=8) as asb,
                tc.tile_pool(name="norm", bufs=2) as nsb,
            ):
                for h in range(HL):
                    ho = h * D
                    for b in range(B):
                        pos = [
                            pso.tile([VS, 512], FP, name=f"po{qc}", tag=f"po{qc}")
                            for qc in range(QT_CH)
                        ]
                        for kt in range(KT_TILES):
                            kcol = b * N + kt * 128
                            vti = (b * N) // 128 + kt
                            vcol = vti * VST * HL + h * VST
                            exs = []
                            for qc in range(QT_CH):
                                qcol = b * N + qc * 512
                                ps = pss.tile([128, 512], FP, name="ps_s", tag="ps_s")
                                nc.tensor.matmul(
                                    ps[:],
                                    lhsT=kT[ho : ho + D, kcol : kcol + 128],
                                    rhs=qT[ho : ho + D, qcol : qcol + 512],
                                    start=True,
                                    stop=True,
                                )
                                ex = asb.tile([128, 512], BF, name="ex", tag="ex")
                                nc.scalar.activation(ex[:], ps[:], AF.Exp, scale=SCALE)
                                exs.append(ex)
                            for qc in range(QT_CH):
                                nc.tensor.matmul(
                                    pos[qc][:],
                                    lhsT=vsb[:, vcol : vcol + VS],
                                    rhs=exs[qc][:],
                                    start=(kt == 0),
                                    stop=(kt == KT_TILES - 1),
                                )
                        for qc in range(QT_CH):
                            # evict psum first so the bank frees for the next
                            # (h, b) pair, then normalize from SBUF
                            poc = nsb.tile([VS, 512], FP, name="poc", tag="poc", bufs=4)
                            nc.vector.tensor_copy(poc[:], pos[qc][:])
                            rc = nsb.tile([1, 512], FP, name="rc", tag="rc")
                            nc.vector.reciprocal(rc[:], poc[D : D + 1, :])
                            bc = nsb.tile([D, 512], FP, name="bc", tag="bc")
                            nc.gpsimd.partition_broadcast(bc[:], rc[:])
                            an = nsb.tile([D, 512], BF, name="an", tag="an")
                            nc.vector.tensor_mul(an[:], poc[0:D, :], bc[:])
                            j = b * QT_CH + qc
                            nc.sync.dma_start(a2a_in[h][j, :, :], an[:])
                    nc.gpsimd.collective_compute(
                        "AllToAll",
                        mybir.AluOpType.bypass,
                        replica_groups=[list(range(NCORES))],
                        ins=[a2a_in[h].opt()],
                        outs=[a2a_out[h].opt()],
                    )

            # ---------------- Phase 3: output projection -------------------
            with (
                tc.tile_pool(name="p3", bufs=1) as p3,
                tc.tile_pool(name="p3y", bufs=2) as p3y,
                tc.tile_pool(name="ps_y", bufs=2, space="PSUM") as psy,
            ):
                wp = p3.tile([128, NCORES * C], BF, name="wp")
                for j in range(NCORES):
                    nc.sync.dma_start(
                        wp[:, j * C : (j + 1) * C], wpT[j * 128 : (j + 1) * 128, :]
                    )
                bp_sb = p3.tile([128, 8], FP, name="bp_sb")
                nc.sync.dma_start(bp_sb[:], bproj)
                # gathered activations: rows 0:64 <- head-0 channels of every
                # rank, rows 64:128 <- head-1 channels (matches wp row order)
                # ga rows 0:64 <- head-0 channels of every rank, 64:128 <- head-1
                ga = p3.tile([128, NCORES * 512], BF, name="ga")
                for h in range(HL):
                    for j in range(NCORES):
                        nc.sync.dma_start(
                            ga[h * D : (h + 1) * D, j * 512 : (j + 1) * 512],
                            a2a_out[h][j],
                        )
                # two independent K=64 accumulation groups per m-tile: the
                # h=0 group closes before A2A(h=1) lands, so it overlaps comm;
                # DVE combines the halves and adds the bias.
                for m in range(C // 128):
                    pys = []
                    for h in range(HL):
                        py = psy.tile([128, 512], FP, name=f"py{h}", tag=f"py{h}")
                        for j in range(NCORES):
                            col = j * C + m * 128
                            nc.tensor.matmul(
                                py[:],
                                lhsT=wp[h * D : (h + 1) * D, col : col + 128],
                                rhs=ga[h * D : (h + 1) * D, j * 512 : (j + 1) * 512],
                                start=(j == 0),
                                stop=(j == NCORES - 1),
                            )
                        pys.append(py)
                    y0 = p3y.tile([128, 512], FP, name="y0", tag="y0")
                    nc.vector.tensor_copy(y0[:], pys[0][:])
                    ysb = p3y.tile([128, 512], FP, name="ysb", tag="ysb")
                    nc.vector.scalar_tensor_tensor(
                        ysb[:], pys[1][:], bp_sb[:, m : m + 1], y0[:],
                        op0=mybir.AluOpType.add, op1=mybir.AluOpType.add,
                    )
                    nc.sync.dma_start(outT[m * 128 : (m + 1) * 128, :], ysb[:])

    nc.compile()
    return nc


def _prep_inputs(inputs):
    """Full inputs -> per-core in_maps (all host-side, cheap reshapes)."""
    x = np.asarray(inputs["x"], dtype=np.float32)
    cos = np.asarray(inputs["cos"], dtype=np.float32)
    sin = np.asarray(inputs["sin"], dtype=np.float32)
    w_qkv = np.asarray(inputs["w_qkv"], dtype=np.float32)
    b_qkv = np.asarray(inputs["b_qkv"], dtype=np.float32)
    w_proj = np.asarray(inputs["w_proj"], dtype=np.float32)
    b_proj = np.asarray(inputs["b_proj"], dtype=np.float32)

    xT = np.ascontiguousarray(x.reshape(T, C).T).astype(BF_NP)
    cosT = cos[0, 0].T  # [64, 2048]
    sinT = sin[0, 0].T.copy()
    sinT[: D // 2] *= -1.0  # fold rotate_half's sign into sin
    cos2 = np.ascontiguousarray(np.tile(cosT, (HL, B))).astype(BF_NP)
    sin2 = np.ascontiguousarray(np.tile(sinT, (HL, B))).astype(BF_NP)
    wpT = np.ascontiguousarray(w_proj.T).astype(BF_NP)
    bp = np.ascontiguousarray(b_proj.reshape(8, 128).T)
    eye = np.eye(128, dtype=np.float32)

    in_maps = []
    for c in range(NCORES):
        rows = np.concatenate(
            [np.arange(g * C + c * CL, g * C + (c + 1) * CL) for g in range(3)]
        )
        wq = np.ascontiguousarray(w_qkv[rows].T).astype(BF_NP)  # [1024, 384]
        bq = np.ascontiguousarray(b_qkv[rows].reshape(3, CL).T)  # [128, 3]
        in_maps.append(
            {
                "xT": xT,
                "wqkvT": wq,
                "bqkv": bq,
                "cos2": cos2,
                "sin2": sin2,
                "wpT": wpT,
                "bproj": bp,
                "eye": eye,
                "ones": np.ones((128, T // 128 * HL), dtype=BF_NP),
            }
        )
    return in_maps


_NC_CACHE = None
last_results = None


def _install_ntff_hook():
    """Best-effort: register the axon NTFF profiling hook that the boot
    skipped (the image's antenv lacks axon_hooks). Trace-mode only."""
    try:
        import types

        if "antenv.axon_hooks" not in sys.modules:
            mod = types.ModuleType("antenv.axon_hooks")
            mod._hook = None
            mod.set_axon_ntff_profile_hook = lambda h: setattr(mod, "_hook", h)
            mod.get_axon_ntff_profile_hook = lambda: mod._hook
            sys.modules["antenv.axon_hooks"] = mod
            import antenv

            antenv.axon_hooks = mod
        import antenv.axon_hooks as ah

        if ah.get_axon_ntff_profile_hook() is None:
            if "/root/.axon_site" not in sys.path:
                sys.path.insert(0, "/root/.axon_site")
            from trn_agent_boot.trn_boot import _ntff_profile_via_ctypes

            hook = _ntff_profile_via_ctypes("/opt/axon/libaxon_pjrt.so")
            if hook is not None:
                ah.set_axon_ntff_profile_hook(hook)
        # artifact upload needs a bucket this sandbox doesn't have
        import concourse.bass_utils as bu

        bu.upload_artifacts = lambda tmpdir: tmpdir
    except Exception as e:  # pragma: no cover - profiling is optional
        print(f"ntff hook install failed: {e}", file=sys.stderr)


def kernel(**inputs):
    global _NC_CACHE, last_results
    from concourse.bass_utils import run_bass_kernel_spmd

    if _NC_CACHE is None:
        _NC_CACHE = _build()
    in_maps = _prep_inputs(inputs)
    trace = os.environ.get("KBENCH_TRACE", "0") == "1"
    if trace:
        _install_ntff_hook()
    res = run_bass_kernel_spmd(
        _NC_CACHE, in_maps, core_ids=list(range(NCORES)), trace=trace
    )
    last_results = res
    shards = [res.results[c]["outT"].T for c in range(NCORES)]  # each [512, 1024]
    y = np.concatenate(shards, axis=0).reshape(B, N, C)
    return np.ascontiguousarray(y.astype(np.float32))


# revision 26
# speedup vs baseline: 1.1246x; 1.0517x over previous
"""Distributed RoPE multi-head attention for one TRN2 chip (8 NeuronCores).

Reference op (B=2, N=2048, C=1024, H=16, D=64, fp32):
    qkv = x @ w_qkv.T + b_qkv ; rope(q), rope(k)
    attn = softmax(q k^T / sqrt(D)) ; out = (attn v) @ w_proj.T + b_proj

Sharding: tensor-parallel over heads. Core c owns heads (2c, 2c+1) for BOTH
batch elements: it computes its slice of the QKV projection, RoPE, and full
attention for its 4 (batch, head) pairs, all in "transposed" layouts
(feature on SBUF partitions, token on the free dim) so no transposes are
needed between the matmuls. An on-chip AllToAll (2 MB/core) then reshards
the attention output from head-sharded to token-sharded, and each core runs
the output projection (full w_proj) + bias for its disjoint 512-token slice.
The host only concatenates the 8 disjoint output shards.

Matmuls run as float32r (full PE rate at free-dim >= 256, fp32 storage).
"""

import os
import sys

import numpy as np

sys.path.insert(0, "/opt/trn_rl_repo")

import ml_dtypes  # noqa: E402

BF_NP = ml_dtypes.bfloat16

import concourse.bacc as bacc  # noqa: E402
import concourse.mybir as mybir  # noqa: E402
import concourse.tile as tile  # noqa: E402

B, N, C, H, D = 2, 2048, 1024, 16, 64
T = B * N                  # 4096 flattened tokens (batch-major)
NCORES = 8
HL = H // NCORES           # 2 heads per core
CL = HL * D                # 128 local channels
TS = T // NCORES           # 512-token output slice per core
SCALE = float(D) ** -0.5
KK = C // 128              # 8 contraction tiles for the qkv matmul
KT_TILES = N // 128        # 16 key tiles per (batch, head)
QT_CH = N // 512           # 4 query chunks of 512 per batch
VS = D + 1                 # v-tile row = 64 v values + a ones column (rowsum)
VST = 80                   # per-head stride in the v tile (16B-aligned for bf16)

FP = mybir.dt.float32
FR = mybir.dt.float32r
BF = mybir.dt.bfloat16
AF = mybir.ActivationFunctionType


def _build():
    nc = bacc.Bacc(
        "TRN2",
        target_bir_lowering=False,
        debug=False,
        enable_asserts=False,
        num_devices=NCORES,
    )

    xT = nc.dram_tensor("xT", [C, T], BF, kind="ExternalInput").ap()
    wqkvT = nc.dram_tensor("wqkvT", [C, 3 * CL], BF, kind="ExternalInput").ap()
    bqkv = nc.dram_tensor("bqkv", [128, 3], FP, kind="ExternalInput").ap()
    cos2 = nc.dram_tensor("cos2", [128, T], BF, kind="ExternalInput").ap()
    sin2 = nc.dram_tensor("sin2", [128, T], BF, kind="ExternalInput").ap()
    wpT = nc.dram_tensor("wpT", [C, C], BF, kind="ExternalInput").ap()
    bproj = nc.dram_tensor("bproj", [128, 8], FP, kind="ExternalInput").ap()
    eye = nc.dram_tensor("eye", [128, 128], FP, kind="ExternalInput").ap()
    ones = nc.dram_tensor("ones", [128, T // 128 * HL], BF, kind="ExternalInput").ap()
    outT = nc.dram_tensor("outT", [C, TS], FP, kind="ExternalOutput").ap()

    with tile.TileContext(nc) as tc:
        with (
            tc.tile_pool(name="persist", bufs=1) as pp,
            tc.tile_pool(name="dram", bufs=1, space="DRAM") as dp,
        ):
            # qT/kT/vT: [2 heads x 64 feature rows, 4096 tokens]
            qT = pp.tile([128, T], BF, name="qT")
            kT = pp.tile([128, T], BF, name="kT")
            vT = pp.tile([128, T], FP, name="vT")
            # v re-tiled token-major: 32 blocks of [128 tokens, 65+65]
            # (64 v features + ones column, per head)
            vsb = pp.tile([128, (T // 128) * VST * HL], BF, name="vsb")
            eye_sb = pp.tile([128, 128], FP, name="eye_sb")
            nc.sync.dma_start(eye_sb[:], eye)

            # ---------------- Phase 1: QKV projection + RoPE + V retile ----
            with (
                tc.tile_pool(name="p1", bufs=1) as p1,
                tc.tile_pool(name="xs", bufs=1) as xs,
                tc.tile_pool(name="ps_qkv", bufs=4, space="PSUM") as ps1,
                tc.tile_pool(name="ps_tr", bufs=2, space="PSUM") as pst,
            ):
                wq = p1.tile([128, KK * 3 * CL], BF, name="wq")
                for kk in range(KK):
                    nc.sync.dma_start(
                        wq[:, kk * 3 * CL : (kk + 1) * 3 * CL],
                        wqkvT[kk * 128 : (kk + 1) * 128, :],
                    )
                bq_sb = p1.tile([128, 3], FP, name="bq_sb")
                nc.sync.dma_start(bq_sb[:], bqkv)

                dma_engines = (nc.sync, nc.gpsimd, nc.scalar)
                xfull = []
                for kk in range(KK):
                    xf = xs.tile([128, T], BF, name="xf", tag=f"xf{kk}")
                    # contiguous 1MB read, spread across engine DMA queues
                    dma_engines[kk % 3].dma_start(
                        xf[:], xT[kk * 128 : (kk + 1) * 128, :]
                    )
                    xfull.append(xf)
                cos_sb = p1.tile([128, T], BF, name="cos_sb")
                sin_sb = p1.tile([128, T], BF, name="sin_sb")
                nc.gpsimd.dma_start(cos_sb[:], cos2)
                nc.scalar.dma_start(sin_sb[:], sin2)
                qkv_dst = (qT, kT, vT)
                for t in range(T // 512):
                    sl = slice(t * 512, (t + 1) * 512)
                    for m in range(3):
                        ps = ps1.tile([128, 512], FP, name="psqkv", tag="psqkv")
                        for kk in range(KK):
                            col = kk * 3 * CL + m * 128
                            nc.tensor.matmul(
                                ps[:],
                                lhsT=wq[:, col : col + 128],
                                rhs=xfull[kk][:, sl],
                                start=(kk == 0),
                                stop=(kk == KK - 1),
                            )
                        nc.vector.tensor_scalar_add(
                            qkv_dst[m][:, sl], ps[:], bq_sb[:, m : m + 1]
                        )

                    # RoPE on this chunk of q and k, in place:
                    #   out = x*cos + rot(x)*sin_signed
                    # rot swaps the d<32 / d>=32 halves within each head's 64
                    # rows (sign folded into sin_signed host-side); partition
                    # moves must go through DMA.
                    for tgt in (qT, kT):
                        t1 = p1.tile([128, 512], FP, name="rope1", tag="rope1", bufs=3)
                        t2 = p1.tile([128, 512], BF, name="rope2", tag="rope2", bufs=3)
                        nc.vector.tensor_mul(t1[:], tgt[:, sl], cos_sb[:, sl])
                        for g in range(HL):
                            o = g * 64
                            nc.sync.dma_start(t2[o : o + 32, :], tgt[o + 32 : o + 64, sl])
                            nc.sync.dma_start(t2[o + 32 : o + 64, :], tgt[o : o + 32, sl])
                        t3 = p1.tile([128, 512], FP, name="rope3", tag="rope3", bufs=3)
                        nc.vector.tensor_mul(t3[:], t2[:], sin_sb[:, sl])
                        nc.vector.tensor_add(tgt[:, sl], t1[:], t3[:])

                    # Retile this chunk of V token-major via PE transpose
                    for tj in range(4):
                        ti = t * 4 + tj
                        pt = pst.tile([128, 128], FP, name="ptr", tag="ptr")
                        nc.tensor.transpose(
                            pt[:], vT[:, ti * 128 : (ti + 1) * 128], eye_sb[:]
                        )
                        base = ti * VST * HL
                        nc.vector.tensor_copy(vsb[:, base : base + D], pt[:, 0:D])
                        nc.vector.tensor_copy(
                            vsb[:, base + VST : base + VST + D], pt[:, D : 2 * D]
                        )
                ones_view = vsb[:].rearrange("p (t c) -> p t c", c=VST)
                nc.sync.dma_start(
                    ones_view[:, :, D : D + 1],
                    ones.rearrange("p (f o) -> p f o", o=1),
                )

            # ---------------- Phase 2: attention + AllToAll + projection ---
            # One AllToAll per local head: A2A(h=0) flies while h=1's
            # attention computes; the h=0 half of the projection is emitted
            # between the two h=1 attention blocks so it lands before A2A(h=1)
            # in the static schedule; the h=1 half overlaps nothing but is
            # small.
            a2a_in = [
                dp.tile([NCORES, D, 512], BF, name=f"a2a_in{h}") for h in range(HL)
            ]
            a2a_out = [
                dp.tile([NCORES, D, 512], BF, name=f"a2a_out{h}") for h in range(HL)
            ]
            with (
                tc.tile_pool(name="ps_s", bufs=3, space="PSUM") as pss,
                tc.tile_pool(name="ps_o", bufs=2, space="PSUM") as pso,
                tc.tile_pool(name="ps_y", bufs=2, space="PSUM") as psy,
                tc.tile_pool(name="exp", bufs=4) as asb,
                tc.tile_pool(name="norm", bufs=2) as nsb,
                tc.tile_pool(name="p3", bufs=1) as p3,
                tc.tile_pool(name="p3y", bufs=2) as p3y,
            ):
                wp = p3.tile([128, NCORES * C], BF, name="wp")
                for j in range(NCORES):
                    nc.sync.dma_start(
                        wp[:, j * C : (j + 1) * C], wpT[j * 128 : (j + 1) * 128, :]
                    )
                bp_sb = p3.tile([128, 8], FP, name="bp_sb")
                nc.sync.dma_start(bp_sb[:], bproj)
                # ga rows 0:64 <- head-0 channels of every rank, 64:128 <- head-1
                ga = p3.tile([128, NCORES * 512], BF, name="ga")
                y0s = []

                def emit_proj_half(h):
                    for j in range(NCORES):
                        nc.sync.dma_start(
                            ga[h * D : (h + 1) * D, j * 512 : (j + 1) * 512],
                            a2a_out[h][j],
                        )
                    for m in range(C // 128):
                        py = psy.tile([128, 512], FP, name="py", tag="py")
                        for j in range(NCORES):
                            col = j * C + m * 128
                            nc.tensor.matmul(
                                py[:],
                                lhsT=wp[h * D : (h + 1) * D, col : col + 128],
                                rhs=ga[h * D : (h + 1) * D, j * 512 : (j + 1) * 512],
                                start=(j == 0),
                                stop=(j == NCORES - 1),
                            )
                        if h == 0:
                            y0 = p3y.tile(
                                [128, 512], FP, name="y0", tag=f"y0_{m}", bufs=1
                            )
                            nc.vector.tensor_copy(y0[:], py[:])
                            y0s.append(y0)
                        else:
                            ysb = p3y.tile([128, 512], FP, name="ysb", tag="ysb")
                            nc.vector.scalar_tensor_tensor(
                                ysb[:], py[:], bp_sb[:, m : m + 1], y0s[m][:],
                                op0=mybir.AluOpType.add, op1=mybir.AluOpType.add,
                            )
                            nc.sync.dma_start(
                                outT[m * 128 : (m + 1) * 128, :], ysb[:]
                            )

                for h in range(HL):
                    ho = h * D
                    for b in range(B):
                        for qc in range(QT_CH):
                            qcol = b * N + qc * 512
                            po = pso.tile([VS, 512], FP, name="po", tag="po")
                            for kt in range(KT_TILES):
                                kcol = b * N + kt * 128
                                vti = (b * N) // 128 + kt
                                vcol = vti * VST * HL + h * VST
                                ps = pss.tile([128, 512], FP, name="ps_s", tag="ps_s")
                                nc.tensor.matmul(
                                    ps[:],
                                    lhsT=kT[ho : ho + D, kcol : kcol + 128],
                                    rhs=qT[ho : ho + D, qcol : qcol + 512],
                                    start=True,
                                    stop=True,
                                )
                                ex = asb.tile([128, 512], BF, name="ex", tag="ex")
                                nc.scalar.activation(ex[:], ps[:], AF.Exp, scale=SCALE)
                                nc.tensor.matmul(
                                    po[:],
                                    lhsT=vsb[:, vcol : vcol + VS],
                                    rhs=ex[:],
                                    start=(kt == 0),
                                    stop=(kt == KT_TILES - 1),
                                )
                            poc = nsb.tile([VS, 512], FP, name="poc", tag="poc", bufs=4)
                            nc.vector.tensor_copy(poc[:], po[:])
                            rc = nsb.tile([1, 512], FP, name="rc", tag="rc")
                            nc.vector.reciprocal(rc[:], poc[D : D + 1, :])
                            bc = nsb.tile([D, 512], FP, name="bc", tag="bc")
                            nc.gpsimd.partition_broadcast(bc[:], rc[:])
                            an = nsb.tile([D, 512], BF, name="an", tag="an")
                            nc.vector.tensor_mul(an[:], poc[0:D, :], bc[:])
                            j = b * QT_CH + qc
                            nc.sync.dma_start(a2a_in[h][j, :, :], an[:])
                        if h == 1 and b == 0:
                            # h=0 projection: ready since A2A(h=0); emitting it
                            # here places it between the h=1 attention blocks
                            emit_proj_half(0)
                    nc.gpsimd.collective_compute(
                        "AllToAll",
                        mybir.AluOpType.bypass,
                        replica_groups=[list(range(NCORES))],
                        ins=[a2a_in[h].opt()],
                        outs=[a2a_out[h].opt()],
                    )
                emit_proj_half(1)

    nc.compile()
    return nc


def _prep_inputs(inputs):
    """Full inputs -> per-core in_maps (all host-side, cheap reshapes)."""
    x = np.asarray(inputs["x"], dtype=np.float32)
    cos = np.asarray(inputs["cos"], dtype=np.float32)
    sin = np.asarray(inputs["sin"], dtype=np.float32)
    w_qkv = np.asarray(inputs["w_qkv"], dtype=np.float32)
    b_qkv = np.asarray(inputs["b_qkv"], dtype=np.float32)
    w_proj = np.asarray(inputs["w_proj"], dtype=np.float32)
    b_proj = np.asarray(inputs["b_proj"], dtype=np.float32)

    xT = np.ascontiguousarray(x.reshape(T, C).T).astype(BF_NP)
    cosT = cos[0, 0].T  # [64, 2048]
    sinT = sin[0, 0].T.copy()
    sinT[: D // 2] *= -1.0  # fold rotate_half's sign into sin
    cos2 = np.ascontiguousarray(np.tile(cosT, (HL, B))).astype(BF_NP)
    sin2 = np.ascontiguousarray(np.tile(sinT, (HL, B))).astype(BF_NP)
    wpT = np.ascontiguousarray(w_proj.T).astype(BF_NP)
    bp = np.ascontiguousarray(b_proj.reshape(8, 128).T)
    eye = np.eye(128, dtype=np.float32)

    in_maps = []
    for c in range(NCORES):
        rows = np.concatenate(
            [np.arange(g * C + c * CL, g * C + (c + 1) * CL) for g in range(3)]
        )
        wq = np.ascontiguousarray(w_qkv[rows].T).astype(BF_NP)  # [1024, 384]
        bq = np.ascontiguousarray(b_qkv[rows].reshape(3, CL).T)  # [128, 3]
        in_maps.append(
            {
                "xT": xT,
                "wqkvT": wq,
                "bqkv": bq,
                "cos2": cos2,
                "sin2": sin2,
                "wpT": wpT,
                "bproj": bp,
                "eye": eye,
                "ones": np.ones((128, T // 128 * HL), dtype=BF_NP),
            }
        )
    return in_maps


_NC_CACHE = None
last_results = None


def _install_ntff_hook():
    """Best-effort: register the axon NTFF profiling hook that the boot
    skipped (the image's antenv lacks axon_hooks). Trace-mode only."""
    try:
        import types

        if "antenv.axon_hooks" not in sys.modules:
            mod = types.ModuleType("antenv.axon_hooks")
            mod._hook = None
            mod.set_axon_ntff_profile_hook = lambda h: setattr(mod, "_hook", h)
            mod.get_axon_ntff_profile_hook = lambda: mod._hook
            sys.modules["antenv.axon_hooks"] = mod
            import antenv

            antenv.axon_hooks = mod
        import antenv.axon_hooks as ah

        if ah.get_axon_ntff_profile_hook() is None:
            if "/root/.axon_site" not in sys.path:
                sys.path.insert(0, "/root/.axon_site")
            from trn_agent_boot.trn_boot import _ntff_profile_via_ctypes

            hook = _ntff_profile_via_ctypes("/opt/axon/libaxon_pjrt.so")
            if hook is not None:
                ah.set_axon_ntff_profile_hook(hook)
        # artifact upload needs a bucket this sandbox doesn't have
        import concourse.bass_utils as bu

        bu.upload_artifacts = lambda tmpdir: tmpdir
    except Exception as e:  # pragma: no cover - profiling is optional
        print(f"ntff hook install failed: {e}", file=sys.stderr)


def kernel(**inputs):
    global _NC_CACHE, last_results
    from concourse.bass_utils import run_bass_kernel_spmd

    if _NC_CACHE is None:
        _NC_CACHE = _build()
    in_maps = _prep_inputs(inputs)
    trace = os.environ.get("KBENCH_TRACE", "0") == "1"
    if trace:
        _install_ntff_hook()
    res = run_bass_kernel_spmd(
        _NC_CACHE, in_maps, core_ids=list(range(NCORES)), trace=trace
    )
    last_results = res
    shards = [res.results[c]["outT"].T for c in range(NCORES)]  # each [512, 1024]
    y = np.concatenate(shards, axis=0).reshape(B, N, C)
    return np.ascontiguousarray(y.astype(np.float32))
